# revision 2
# baseline (speedup 1.0000x reference)
"""Trainium2 Bass kernel: GQA attention (B=2, S=1024, dim=2048, 32 Q / 8 KV heads).

Sharding: 2 batches x 4 kv-head-pairs over 8 cores.  Core c owns batch c//4
and kv heads {2m, 2m+1} (m = c%4) with their 8 query heads.  Each core reads
its batch's x (transposed) and produces a partial [1024, 2048] bf16 output;
the host sums 4 partials per batch.

Device dataflow per core (1024 tokens = 8 tiles):
  kv-first QKV: x^T tiles @ wqkv (col-sharded, rope-permuted) -> PSUM
  RoPE on DVE, PE-transpose to d-major qT/kT; V kept bf16 key tiles
  scores^T = k^T.T @ q^T, two heads packed via PE row groups
  exp on ACT (scale 1/8 folded) -> bf16
  attn@V matmuls with a ones column giving softmax row sums in PSUM row 64
  reciprocal on DVE, broadcast across partitions via K=1 PE matmul
  normalize during PSUM evict -> aoT bf16
  out^T partial = aoT.T @ wo shard -> bf16 -> DRAM

Scheduling: the softmax exp on ACT (1114ns per [128,1024] tile) is longer
than one kc-step of scores+attnV on PE (852ns), so the q-projection
(phase B) and out-projection (phase C) matmuls are interleaved 2-per-kc
into the attention loop to keep the PE ahead of ACT.  Input DMAs are
issued in consumption order with the first tensors chunked so the first
matmul can start as soon as the first chunks land.
"""

import os
import sys
import numpy as np
from contextlib import ExitStack

sys.path.insert(0, "/opt/trn_rl_repo")

import concourse.bass as bass
import concourse.tile as tile
from concourse import bacc
from concourse import mybir
from concourse.bass_utils import run_bass_kernel_spmd


B, S, DIM = 2, 1024, 2048
NCORES = 8
D = 64
TPC = 1024                    # tokens per core (one batch)
NT = TPC // 128               # 8 token tiles
NPAIR = 4                     # q-head pairs per core (a_i, b_i)
QCOLS = 8 * D                 # 512 q cols
WCOLS = QCOLS + 2 * D + 2 * D  # 768 (q8 | kA kB | vA vB)
ROPE_THETA = 10000.0
SCALE = 1.0 / float(np.sqrt(D))

F32 = mybir.dt.float32
BF16 = mybir.dt.bfloat16
MUL = mybir.AluOpType.mult
ADD = mybir.AluOpType.add
SUB = mybir.AluOpType.subtract
EXP = mybir.ActivationFunctionType.Exp


def _build():
    nc = bacc.Bacc(
        "TRN2",
        target_bir_lowering=False,
        debug=False,
        num_devices=NCORES,
    )
    xq_d = [nc.dram_tensor("xq%d" % q, [128, 16 * 256], BF16,
                           kind="ExternalInput").ap() for q in range(4)]
    wkv_d = nc.dram_tensor("wkv", [128, 16 * 256], BF16, kind="ExternalInput").ap()
    wq_d = nc.dram_tensor("wq", [128, 16 * 512], BF16, kind="ExternalInput").ap()
    wo_d = nc.dram_tensor("wo", [128, 4 * DIM], BF16, kind="ExternalInput").ap()
    cos_d = nc.dram_tensor("cosb", [128, 256], F32, kind="ExternalInput").ap()
    sin_d = nc.dram_tensor("sinb", [128, 256], F32, kind="ExternalInput").ap()
    id_d = nc.dram_tensor("ident", [128, 128], BF16, kind="ExternalInput").ap()
    outp = nc.dram_tensor("out", [TPC, DIM], BF16, kind="ExternalOutput").ap()

    with tile.TileContext(nc) as tc, ExitStack() as ctx:
        p = lambda name, bufs, space="SBUF": ctx.enter_context(
            tc.tile_pool(name=name, bufs=bufs, space=space)
        )
        p_const = p("const", 1)
        p_w = p("wqkv", 1)
        p_wo = p("wo", 1)
        p_cs = p("cs", 2)
        p_xt = p("xt", 1)
        p_qrot = p("qrot", 3)
        p_krot = p("krot", 2)
        p_ta = p("ta", 3)
        p_tb = p("tb", 3)
        p_qT = p("qT", 4)
        p_kT = p("kT", 1)
        p_vp = p("vp", 16)
        p_es = p("es", 3)
        p_ao = p("ao", 4)
        p_rr = p("rr", 4)
        p_bc = p("bc", 2)
        p_osb = p("osb", 2)
        # PSUM: 4 + 2 + 2 banks
        p_big = p("big", 2, space="PSUM")      # qkv + scores [128,1024] f32
        p_at = p("at", 2, space="PSUM")        # AV accum [65,512] f32
        p_mix = p("mix", 2, space="PSUM")      # transpose/bc/outproj

        ident = p_const.tile([128, 128], BF16, tag="ident", name="ident")
        cosc = p_cs.tile([128, 256], F32, tag="cosc", name="cosc")
        sinc = p_cs.tile([128, 256], F32, tag="sinc", name="sinc")
        nc.sync.dma_start(cosc[:], cos_d[:])
        nc.sync.dma_start(sinc[:], sin_d[:])
        nc.sync.dma_start(ident[:], id_d[:])

        # all inputs host-packed to SBUF layout: straight full-rate copies.
        # Issue order == consumption order; the tensors needed first (wkv,
        # xq0, wq) are chunked so compute can start on the first chunks.
        xtq = [p_xt.tile([128, 16 * 256], BF16, tag="xtq%d" % q4, name="xtq")
               for q4 in range(4)]
        wkv_sb = p_w.tile([128, 16 * 256], BF16, tag="wkv", name="wkv")
        wq_sb = p_w.tile([128, 16 * 512], BF16, tag="wq", name="wq")
        wo_big = p_wo.tile([128, 4 * DIM], BF16, tag="wo", name="wo")
        for c in range(4):
            sl = slice(c * 1024, (c + 1) * 1024)
            nc.sync.dma_start(wkv_sb[:, sl], wkv_d[:, sl])
            nc.sync.dma_start(xtq[0][:, sl], xq_d[0][:, sl])
        for c in range(4):
            sl = slice(c * 2048, (c + 1) * 2048)
            nc.sync.dma_start(wq_sb[:, sl], wq_d[:, sl])
        nc.sync.dma_start(xtq[1][:], xq_d[1][:])
        nc.sync.dma_start(xtq[2][:], xq_d[2][:])
        nc.sync.dma_start(xtq[3][:], xq_d[3][:])
        nc.sync.dma_start(wo_big[:], wo_d[:])

        def xt_ap(kd, tt):
            q4, r = tt // 2, tt % 2
            return xtq[q4][:, kd * 256 + r * 128:kd * 256 + (r + 1) * 128]

        wkv_ap = lambda kd: wkv_sb[:, kd * 256:(kd + 1) * 256]
        wqc_ap = lambda kd: wq_sb[:, kd * 512:(kd + 1) * 512]
        wo_ap = lambda hp, oc: wo_big[:, hp * DIM + oc * 512:hp * DIM + (oc + 1) * 512]
        # expand compact rope tables to one [128, 256] block per head slot
        # (kv slots 8,9 first: the first kv tile's rope needs them earliest)
        cos_t = p_cs.tile([128, 2560], F32, tag="cos", name="cos")
        sin_t = p_cs.tile([128, 2560], F32, tag="sin", name="sin")
        for h in (8, 9, 0, 1, 2, 3, 4, 5, 6, 7):
            nc.scalar.copy(cos_t[:, h * 256:(h + 1) * 256], cosc[:])
            nc.scalar.copy(sin_t[:, h * 256:(h + 1) * 256], sinc[:])

        ones = p_const.tile([1, 64], BF16, tag="ones", name="ones")
        nc.vector.memset(ones[:], 1.0)
        # warm the ACT exp table early (dummy activation on a ready tile)
        warm = p_const.tile([1, 8], F32, tag="warm", name="warm")
        nc.vector.memset(warm[:], 0.0)
        nc.scalar.activation(warm[:], warm[:], EXP)

        qT = [p_qT.tile([128, TPC], BF16, tag="qT", name="qT") for _ in range(NPAIR)]
        kT = p_kT.tile([128, TPC], BF16, tag="kT", name="kT")
        aoT = [p_ao.tile([128, TPC], BF16, tag="aoT", name="aoT")
               for _ in range(NPAIR)]
        # V tiles: [128 keys, 65] (cols 0:64 = v, col 64 = ones for row sums)
        vpA = [p_vp.tile([128, 65], BF16, tag="vp", name="vp") for _ in range(NT)]
        vpB = [p_vp.tile([128, 65], BF16, tag="vp", name="vp") for _ in range(NT)]
        for kc in range(NT):
            nc.vector.memset(vpA[kc][:, 64:65], 1.0)
            nc.vector.memset(vpB[kc][:, 64:65], 1.0)

        cs_view = lambda t: t.rearrange("q (g t i) -> q g t i", g=10, t=8)

        def rope(ps_cols, nheads, sl0, tt, out_bf):
            # ps_cols: PSUM AP [128, nheads*64] (x1|x2 halves per head)
            g = lambda ap: ap.rearrange("q (g i) -> q g i", g=nheads)
            def eo(ap, e):
                v = ap.rearrange("q (g e i) -> q g e i", g=nheads, e=2, i=32)
                return v[:, :, e, :]
            w = nheads * 32
            cosv = cs_view(cos_t)[:, sl0:sl0 + nheads, tt, :]
            sinv = cs_view(sin_t)[:, sl0:sl0 + nheads, tt, :]
            x1, x2 = eo(ps_cols, 0), eo(ps_cols, 1)
            t1 = p_ta.tile([128, 256], F32, tag="ta", name="ta")
            t2 = p_tb.tile([128, 256], F32, tag="tb", name="tb")
            t3 = p_ta.tile([128, 256], F32, tag="ta", name="ta")
            t4 = p_tb.tile([128, 256], F32, tag="tb", name="tb")
            nc.vector.tensor_tensor(g(t1[:, 0:w]), x1, cosv, MUL)
            nc.vector.tensor_tensor(g(t2[:, 0:w]), x2, sinv, MUL)
            nc.vector.tensor_tensor(g(t3[:, 0:w]), x1, sinv, MUL)
            nc.vector.tensor_tensor(g(t4[:, 0:w]), x2, cosv, MUL)
            nc.vector.tensor_tensor(eo(out_bf, 0), g(t1[:, 0:w]), g(t2[:, 0:w]), SUB)
            nc.vector.tensor_tensor(eo(out_bf, 1), g(t3[:, 0:w]), g(t4[:, 0:w]), ADD)

        def kv_tile(tt, evict):
            ps = p_big.tile([128, 1024], F32, tag="big", name="big")
            for kd in range(16):
                nc.tensor.matmul(
                    ps[:, 0:256],
                    xt_ap(kd, tt),
                    wkv_ap(kd),
                    start=(kd == 0),
                    stop=(kd == 15),
                )
            krot = p_krot.tile([128, 128], BF16, tag="krot", name="krot")
            rope(ps[:, 0:128], 2, 8, tt, krot[:])
            nc.vector.tensor_copy(vpA[tt][:, 0:64], ps[:, 128:192])
            nc.vector.tensor_copy(vpB[tt][:, 0:64], ps[:, 192:256])
            ps_tr = p_mix.tile([128, 640], BF16, tag="mix", name="mix")
            nc.tensor.transpose(ps_tr[:, 0:128], krot[:], ident[:])
            evict(kT[:, tt * 128:(tt + 1) * 128], ps_tr[:, 0:128])

        def q_tile_steps(tt, evict, ps_pool, ps_shape):
            """16 q-projection matmuls as 8 kc-step thunk pairs + finish."""
            holder = {}

            def mm(kd):
                def f():
                    if kd == 0:
                        holder["ps"] = ps_pool.tile(
                            ps_shape, F32, tag=ps_pool.name, name=ps_pool.name)
                    nc.tensor.matmul(
                        holder["ps"][:, 0:512],
                        xt_ap(kd, tt),
                        wqc_ap(kd),
                        start=(kd == 0),
                        stop=(kd == 15),
                    )
                return f

            steps = [[mm(2 * k), mm(2 * k + 1)] for k in range(8)]

            def finish():
                ps = holder["ps"]
                qrot = p_qrot.tile([128, 512], BF16, tag="qrot", name="qrot")
                rope(ps[:, 0:512], 8, 0, tt, qrot[:])
                ps_tr = p_mix.tile([128, 640], BF16, tag="mix", name="mix")
                for c in range(4):
                    nc.tensor.transpose(
                        ps_tr[:, c * 128:(c + 1) * 128],
                        qrot[:, c * 128:(c + 1) * 128],
                        ident[:],
                    )
                for c in range(4):
                    evict(qT[c][:, tt * 128:(tt + 1) * 128],
                          ps_tr[:, c * 128:(c + 1) * 128])
            return steps, [finish]

        def outproj_steps(tt, evict):
            """16 out-proj matmuls + evicts + out DMA as kc-step thunks."""
            osb = p_osb.tile([128, DIM], BF16, tag="osb", name="osb")
            ps = {}

            def mm(oc, hp):
                def f():
                    if hp == 0:
                        ps[oc] = p_mix.tile([128, 512], F32, tag="mix", name="mix")
                    nc.tensor.matmul(
                        ps[oc][:],
                        aoT[hp][:, tt * 128:(tt + 1) * 128],
                        wo_ap(hp, oc),
                        start=(hp == 0),
                        stop=(hp == 3),
                    )
                return f

            def ev(oc):
                def f():
                    evict(osb[:, oc * 512:(oc + 1) * 512], ps[oc][:])
                return f

            def dma_half(h):
                def f():
                    nc.gpsimd.dma_start(
                        outp[tt * 128:(tt + 1) * 128, h * 1024:(h + 1) * 1024],
                        osb[:, h * 1024:(h + 1) * 1024])
                return f

            steps = [
                [mm(0, 0), mm(0, 1)],
                [mm(0, 2), mm(0, 3)],
                [ev(0), mm(1, 0), mm(1, 1)],
                [mm(1, 2), mm(1, 3)],
                [ev(1), dma_half(0), mm(2, 0), mm(2, 1)],
                [mm(2, 2), mm(2, 3)],
                [ev(2), mm(3, 0), mm(3, 1)],
                [mm(3, 2), mm(3, 3)],
            ]
            tail = [ev(3), dma_half(1)]
            return steps, tail

        def group_body(i, qc, steps=None, tail=None):
            psA = p_at.tile([65, 512], F32, tag="at", name="at")
            psB = p_at.tile([65, 512], F32, tag="at", name="at")
            es_prev = None
            for kc in range(NT):
                ps_s = p_big.tile([128, 1024], F32, tag="big", name="big")
                nc.tensor.matmul(
                    ps_s[:, 0:512],
                    kT[0:64, kc * 128:(kc + 1) * 128],
                    qT[i][0:64, qc * 512:(qc + 1) * 512],
                    start=True, stop=True,
                    tile_position=(0, 0),
                )
                nc.tensor.matmul(
                    ps_s[:, 512:1024],
                    kT[64:128, kc * 128:(kc + 1) * 128],
                    qT[i][64:128, qc * 512:(qc + 1) * 512],
                    start=True, stop=True,
                    tile_position=(64, 0),
                )
                es = p_es.tile([128, 1024], BF16, tag="es", name="es")
                nc.scalar.activation(es[:], ps_s[:], EXP, scale=SCALE)
                if kc > 0:
                    kp, ep = es_prev
                    nc.tensor.matmul(psA[:], vpA[kp][:], ep[:, 0:512],
                                     start=(kp == 0), stop=False)
                    nc.tensor.matmul(psB[:], vpB[kp][:], ep[:, 512:1024],
                                     start=(kp == 0), stop=False)
                if steps is not None:
                    for t in steps[kc]:
                        t()
                es_prev = (kc, es)
            kp, ep = es_prev
            nc.tensor.matmul(psA[:], vpA[kp][:], ep[:, 0:512],
                             start=False, stop=True)
            nc.tensor.matmul(psB[:], vpB[kp][:], ep[:, 512:1024],
                             start=False, stop=True)
            for t in (tail or []):
                t()
            # stage softmax sums row early so the DVE has it ready by finish
            srow = p_rr.tile([1, 1024], BF16, tag="rr", name="rr")
            nc.vector.tensor_copy(srow[:, 0:512], psA[64:65, :])
            nc.vector.tensor_copy(srow[:, 512:1024], psB[64:65, :])
            return psA, psB, srow

        def group_finish(i, qc, psA, psB, srow):
            ps_bc = p_mix.tile([128, 512], F32, tag="mix", name="mix")
            nc.tensor.matmul(ps_bc[0:64, :], ones[:], srow[:, 0:512],
                             start=True, stop=True, tile_position=(0, 0))
            nc.tensor.matmul(ps_bc[64:128, :], ones[:], srow[:, 512:1024],
                             start=True, stop=True, tile_position=(0, 64))
            bc_sb = p_bc.tile([128, 512], F32, tag="bc", name="bc")
            nc.vector.reciprocal_approx_fast(bc_sb[:], ps_bc[:])
            nc.vector.tensor_tensor(
                aoT[i][0:64, qc * 512:(qc + 1) * 512],
                psA[0:64, :], bc_sb[0:64, :], MUL)
            nc.vector.tensor_tensor(
                aoT[i][64:128, qc * 512:(qc + 1) * 512],
                psB[0:64, :], bc_sb[64:128, :], MUL)

        def outproj_tail(tt, ev_a, ev_b):
            """Phase D out-proj: hp0-2 first (no dep on the last group),
            hp3 + evict + DMA after; evicts split across two engines."""
            osb = p_osb.tile([128, DIM], BF16, tag="osb", name="osb")
            ps = {}

            def mm3(oc):
                ps[oc] = p_mix.tile([128, 512], F32, tag="mix", name="mix")
                for hp in range(3):
                    nc.tensor.matmul(
                        ps[oc][:],
                        aoT[hp][:, tt * 128:(tt + 1) * 128],
                        wo_ap(hp, oc),
                        start=(hp == 0), stop=False)

            def mmL(oc):
                nc.tensor.matmul(
                    ps[oc][:],
                    aoT[3][:, tt * 128:(tt + 1) * 128],
                    wo_ap(3, oc),
                    start=False, stop=True)

            def dma_half(h):
                nc.gpsimd.dma_start(
                    outp[tt * 128:(tt + 1) * 128, h * 1024:(h + 1) * 1024],
                    osb[:, h * 1024:(h + 1) * 1024])

            mm3(0)
            mm3(1)
            mmL(0)
            ev_a(osb[:, 0:512], ps[0][:])
            mm3(2)
            mmL(1)
            ev_b(osb[:, 512:1024], ps[1][:])
            dma_half(0)
            mm3(3)
            mmL(2)
            ev_a(osb[:, 1024:1536], ps[2][:])
            mmL(3)
            ev_b(osb[:, 1536:2048], ps[3][:])
            dma_half(1)

        act_ev = nc.scalar.copy
        dve_ev = nc.vector.tensor_copy

        # Phase A: kv/q interleaved in DMA-supply order (ACT idle -> ACT evicts)
        kv_tile(0, act_ev)
        kv_tile(1, act_ev)
        for tt in range(2):
            st, fin = q_tile_steps(tt, act_ev, p_big, [128, 1024])
            for s in st:
                for t in s:
                    t()
            fin[0]()
        kv_tile(2, act_ev)
        kv_tile(3, act_ev)
        for tt in range(2, 4):
            st, fin = q_tile_steps(tt, act_ev, p_big, [128, 1024])
            for s in st:
                for t in s:
                    t()
            fin[0]()
        for tt in range(4, NT):
            kv_tile(tt, act_ev)
        # Phase B: qc0 groups with the remaining q tiles interleaved per-kc
        for i in range(NPAIR):
            steps, fin = q_tile_steps(4 + i, dve_ev, p_mix, [128, 512])
            st = group_body(i, 0, steps, fin)
            group_finish(i, 0, *st)
        # Phase C: qc1 groups with the first out-proj tiles interleaved per-kc
        for i in range(NPAIR):
            steps, tail = outproj_steps(i, dve_ev)
            st = group_body(i, 1, steps, tail)
            group_finish(i, 1, *st)
        # Phase D: tail outproj (ACT free again); hp0-2 issue before the
        # last group's normalization completes, keeping the PE busy
        for tt in range(4, NT):
            outproj_tail(tt, act_ev, dve_ev)
    nc.compile()
    return nc


_CACHE = {}


def _get_program():
    if "nc" not in _CACHE:
        _CACHE["nc"] = _build()
    return _CACHE["nc"]


def host_inputs(x, wq, wk, wv, wo):
    """Host-side prep: transpose x, shard+permute weights, rope tables."""
    import ml_dtypes
    bf16 = ml_dtypes.bfloat16
    x = np.asarray(x, dtype=np.float32)
    wq = np.asarray(wq, dtype=np.float32)
    wk = np.asarray(wk, dtype=np.float32)
    wv = np.asarray(wv, dtype=np.float32)
    wo = np.asarray(wo, dtype=np.float32)
    perm = np.concatenate([np.arange(0, D, 2), np.arange(1, D, 2)])
    inv_freq = 1.0 / (ROPE_THETA ** (np.arange(0, D, 2, dtype=np.float64) / D))
    ang = np.arange(S, dtype=np.float64)[:, None] * inv_freq[None, :]
    cosb = np.ascontiguousarray(
        np.cos(ang).astype(np.float32).reshape(NT, 128, 32)
        .transpose(1, 0, 2).reshape(128, NT * 32))
    sinb = np.ascontiguousarray(
        np.sin(ang).astype(np.float32).reshape(NT, 128, 32)
        .transpose(1, 0, 2).reshape(128, NT * 32))
    ident = np.eye(128).astype(bf16)

    def pack(a, nblk):
        # [nblk*128, C] row-tiled -> [128, nblk*C] SBUF layout
        C = a.shape[1]
        return np.ascontiguousarray(
            a.reshape(nblk, 128, C).transpose(1, 0, 2).reshape(128, nblk * C)
            .astype(bf16))

    xqb = []
    for b in range(B):
        x3 = x[b].T.reshape(16, 128, TPC).transpose(1, 0, 2)  # [128, 16, 1024]
        xqb.append([
            np.ascontiguousarray(
                x3[:, :, q * 256:(q + 1) * 256].reshape(128, 16 * 256)
                .astype(bf16))
            for q in range(4)])
    in_maps = []
    for c in range(NCORES):
        bc, m = c // 4, c % 4
        qh = [8 * m + i for i in range(4)]        # a_i
        qh2 = [8 * m + 4 + i for i in range(4)]   # b_i
        cols = []
        for i in range(4):
            cols.append(wq[:, qh[i] * D:(qh[i] + 1) * D][:, perm])
            cols.append(wq[:, qh2[i] * D:(qh2[i] + 1) * D][:, perm])
        wq_c = np.concatenate(cols, axis=1)
        gA, gB = 2 * m, 2 * m + 1
        wkv_c = np.concatenate([
            wk[:, gA * D:(gA + 1) * D][:, perm],
            wk[:, gB * D:(gB + 1) * D][:, perm],
            wv[:, gA * D:(gA + 1) * D],
            wv[:, gB * D:(gB + 1) * D],
        ], axis=1)
        rows = []
        for i in range(4):
            rows.append(wo[qh[i] * D:(qh[i] + 1) * D, :])
            rows.append(wo[qh2[i] * D:(qh2[i] + 1) * D, :])
        wo_c = np.concatenate(rows, axis=0)
        in_maps.append(
            {
                "xq0": xqb[bc][0],
                "xq1": xqb[bc][1],
                "xq2": xqb[bc][2],
                "xq3": xqb[bc][3],
                "wkv": pack(wkv_c, 16),
                "wq": pack(wq_c, 16),
                "wo": pack(wo_c, 4),
                "cosb": cosb,
                "sinb": sinb,
                "ident": ident,
            }
        )
    return in_maps


def kernel(x, wq, wk, wv, wo):
    nc = _get_program()
    in_maps = host_inputs(x, wq, wk, wv, wo)
    trace = bool(int(os.environ.get("KERNEL_TRACE", "0")))
    import time as _time
    _t0 = _time.time()
    res = run_bass_kernel_spmd(nc, in_maps, list(range(NCORES)), trace=trace)
    _CACHE["run_wall_s"] = _time.time() - _t0
    _CACHE["last_results"] = res
    out = np.zeros((B, TPC, DIM), dtype=np.float32)
    for c in range(NCORES):
        out[c // 4] += np.asarray(res.results[c]["out"], dtype=np.float32)
    return out
